# revision 21
# baseline (speedup 1.0000x reference)
"""Trainium2 Bass kernel for nn_DecFormerT1 (dense transformer block).

Computation (see problem reference):
  x [2, 8, 128, 24, 24] ->
  qkv projections (+ sine pos embed on q,k) -> full softmax attention over
  n = t*h*w = 4608 -> residual -> channels-first LayerNorm -> grouped-conv
  3x3 FFN (128 -> 512 -> 128, 32 groups) with relu -> residual -> LayerNorm.

Sharding over 8 cores: core j handles batch j//4, query/FFN t-slice
[2*(j%4), 2*(j%4)+2).  K/V are recomputed per-core for the full sequence
(cheap: 2 x [4608,128]@[128,128]) so no collectives are needed.

On-chip layout is channels-first: activations live as [c=128 partitions,
positions] tiles.  Attention uses S^T blocks ([nk, nq], softmax along
partitions via ones-matmul rowsums), P@V accumulates O^T = V^T P^T per
nq-tile of 384, and the FFN does LayerNorm-over-partitions via GPSIMD
partition_all_reduce + grouped conv as 9 shifted matmuls on zero-padded
[c, 26*26] image tiles with block-diagonal dense weights.

Scheduling: input DMAs are chunked per-t and FFN weights prefetched up
front; the attention inner loop is software-pipelined (next tri-group's
S matmuls are emitted before this tri-group's P@V so the exp latency is
hidden); LayerNorms run per-image and overlap the attention tail / conv
of the other image.

Matmuls run in float32r (tf32-like, ~1e-3 rel err, 4x faster than fp32).
"""

from contextlib import ExitStack

import ml_dtypes
import numpy as np

import concourse.bass as bass
import concourse.tile as tile
from concourse import bacc, mybir
from concourse.bass_utils import run_bass_kernel_spmd

F32 = mybir.dt.float32
F32R = mybir.dt.float32r
BF16 = mybir.dt.bfloat16

B, T, C, H, W = 2, 8, 128, 24, 24
HW = H * W  # 576
N = T * HW  # 4608
TPC = 2  # t per core
NQ = TPC * HW  # 1152
NCORES = 8
GROUPS = 32
CH = 4 * C  # 512
EPS = 1e-6
TEMP = 10000.0

NQT = 384  # nq tile for attention
NOQ = NQ // NQT  # 3
NKB = N // 128  # 36 key blocks
NTRI = NKB // 3  # 12 tri-groups

ALU = mybir.AluOpType
ACTF = mybir.ActivationFunctionType


def _pos_embed_np():
    """PositionEmbeddingSine3D closed form, separable: returns
    (pos_yx [HW, C], pz [T, C]); pos[t*HW+s] = pos_yx[s] + pz[t]."""
    npf = C // 2
    scale = 2.0 * np.pi

    def sine(coord, nf):
        dim_t = (TEMP ** (2.0 * (np.arange(nf) // 2).astype(np.float32) / nf)).astype(
            np.float32
        )
        p = coord[:, None] / dim_t  # [L, nf]
        return np.stack(
            [np.sin(p[:, 0::2]), np.cos(p[:, 1::2])], axis=-1
        ).reshape(coord.shape[0], nf)

    z = (np.arange(1, T + 1, dtype=np.float32) / np.float32(T + EPS)) * np.float32(
        scale
    )
    y = (np.arange(1, H + 1, dtype=np.float32) / np.float32(H + EPS)) * np.float32(
        scale
    )
    x = (np.arange(1, W + 1, dtype=np.float32) / np.float32(W + EPS)) * np.float32(
        scale
    )
    pz = sine(z, 2 * npf)  # [T, C]
    py = sine(y, npf)  # [H, npf]
    px = sine(x, npf)  # [W, npf]
    pos_yx = np.empty((H, W, C), dtype=np.float32)
    pos_yx[..., :npf] = py[:, None, :]
    pos_yx[..., npf:] = px[None, :, :]
    return pos_yx.reshape(HW, C), pz


def build_program(reps: int = 1) -> bacc.Bacc:
    nc = bacc.Bacc("TRN2", target_bir_lowering=False, debug=False, num_devices=NCORES)

    def din(name, shape, dt=F32):
        return nc.dram_tensor(name, shape, dt, kind="ExternalInput").ap()

    # per-core data
    xb_r = din("xb_r", [T, C, HW], F32R)  # batch for k (f32r path)
    xb_bf = din("xb_bf", [T, C, HW], BF16)  # batch for v (bf16 path)
    xq_r = din("xq_r", [TPC, C, HW], F32R)  # slice for q
    xq_f = din("xq_f", [TPC, C, HW], F32)  # slice for residual (+bv folded)
    # separable position embedding: pos[t*HW+s, c] = posyx[s, c] + pz[t, c]
    posyx = din("posyx", [C, HW])  # yx part (+nothing), for k
    pzk = din("pzk", [C, T])  # pz + bk, for k
    posqx = din("posqx", [C, HW])  # yx part * isq, for q
    pzq = din("pzq", [C, TPC])  # (pz_slice + bq) * isq, per-core
    wqT = din("wqT", [C, C], F32R)  # Wq.T / sqrt(C)
    wkT = din("wkT", [C, C], F32R)
    wvT = din("wvT", [C, C], BF16)
    ones_r = din("ones_r", [C, 1], F32R)
    w1 = din("w1", [C, 9, C], F32R)  # conv1 lhsT [ic, tap, oc_within_chunk]
    b1 = din("b1", [C, 4])
    w2 = din("w2", [C, 9, 4, C], F32R)  # conv2 lhsT [icw, tap, icchunk, oc]
    b2 = din("b2", [C, 1])
    n1w = din("n1w", [C, 1])
    n1b = din("n1b", [C, 1])
    n2w = din("n2w", [C, 1])
    n2b = din("n2b", [C, 1])

    out = nc.dram_tensor("out", [TPC, C, HW], F32, kind="ExternalOutput").ap()

    with tile.TileContext(nc) as tc:
        for _rep in range(reps):
            _emit_body(
                nc, tc, xb_r, xb_bf, xq_r, xq_f, posyx, pzk, posqx, pzq,
                wqT, wkT, wvT, ones_r, w1, b1, w2, b2, n1w, n1b, n2w, n2b,
                out, chain=(_rep > 0),
            )

    nc.compile()
    return nc


def _emit_body(
    nc, tc, xb_r, xb_bf, xq_r, xq_f, posyx, pzk, posqx, pzq, wqT, wkT, wvT,
    ones_r, w1, b1, w2, b2, n1w, n1b, n2w, n2b, out,
    chain=False,
):
    with ExitStack() as octx:
        # long-lived pools (span attention phases)
        keep = octx.enter_context(tc.tile_pool(name="keep", bufs=1))
        consts = octx.enter_context(tc.tile_pool(name="consts", bufs=1))
        abpool = octx.enter_context(tc.tile_pool(name="abpool", bufs=1))

        # -------- DMA issue order: attention-critical tensors first --------
        wk = consts.tile([C, C], F32R)
        nc.sync.dma_start(wk, wkT)
        wq = consts.tile([C, C], F32R)
        nc.sync.dma_start(wq, wqT)

        with ExitStack() as actx:
            apool = actx.enter_context(tc.tile_pool(name="apool", bufs=1))
            # dma_start issue costs ~1.1us of the issuing engine's sequencer;
            # spread the input DMAs across idle sequencers (only SP/ACT/POOL
            # can issue) so phase A isn't bottlenecked on serial descriptor
            # generation.
            xb = apool.tile([C, T, HW], F32R)
            xbv = apool.tile([C, T, HW], BF16)
            posyxt = apool.tile([C, HW], F32)
            nc.gpsimd.dma_start(posyxt, posyx)
            for t in range(T):
                eng = nc.gpsimd if t % 2 == 0 else nc.scalar
                eng.dma_start(xb[:, t, :], xb_r[t])
            for t in range(T):
                eng = nc.gpsimd if t % 2 == 0 else nc.scalar
                eng.dma_start(xbv[:, t, :], xb_bf[t])
            xq = apool.tile([C, TPC, HW], F32R)
            posqxt = apool.tile([C, HW], F32)
            pzkt = consts.tile([C, T], F32)
            pzqt = consts.tile([C, TPC], F32)
            wv = consts.tile([C, C], BF16)
            nc.sync.dma_start(wv, wvT)
            nc.sync.dma_start(pzkt, pzk)
            for t in range(TPC):
                nc.sync.dma_start(xq[:, t, :], xq_r[t])
            nc.sync.dma_start(posqxt, posqx)
            nc.sync.dma_start(pzqt, pzq)
            onesr = consts.tile([C, 1], F32R)
            nc.sync.dma_start(onesr, ones_r)

            xqf = keep.tile([C, TPC, HW], F32)
            nc.sync.dma_start(xqf, xq_f.rearrange("t c s -> c t s"))
            if chain:
                # benign data dependency on the previous rep's output (timing
                # builds only): xqf += 0 * prev_out — forces serialization and
                # defeats any downstream dead-code elimination of earlier reps.
                prev = keep.tile([C, TPC, HW], F32)
                nc.sync.dma_start(prev, out.rearrange("t c s -> c t s"))
                nc.vector.scalar_tensor_tensor(
                    out=xqf, in0=prev, scalar=0.0, in1=xqf,
                    op0=ALU.mult, op1=ALU.add,
                )

            # FFN weights: prefetch now, consumed ~60us later
            w1t = consts.tile([C, 9, C], F32R)
            nc.scalar.dma_start(w1t, w1)
            w2t = consts.tile([C, 9, 4, C], F32R)
            nc.scalar.dma_start(w2t, w2)
            b1t = consts.tile([C, 4], F32)
            nc.scalar.dma_start(b1t, b1)
            b2t = consts.tile([C, 1], F32)
            nc.scalar.dma_start(b2t, b2)
            n1wt = consts.tile([C, 1], F32)
            nc.scalar.dma_start(n1wt, n1w)
            n1bt = consts.tile([C, 1], F32)
            nc.scalar.dma_start(n1bt, n1b)
            n2wt = consts.tile([C, 1], F32)
            nc.scalar.dma_start(n2wt, n2w)
            n2bt = consts.tile([C, 1], F32)
            nc.scalar.dma_start(n2bt, n2b)
            epst = consts.tile([C, 1], F32)
            nc.vector.memset(epst, EPS)

            ot_sb = keep.tile([C, NQ], F32)  # attention out (normalized)
            kT = abpool.tile([C, N], F32R)
            vb = abpool.tile([C, NKB, C], F32R)  # v blocks [nk, c] (transposed)
            qT = abpool.tile([C, NQ], F32R)

            # ---------------- phase A: projections + v transpose ----------------
            apsum = actx.enter_context(
                tc.tile_pool(name="apsum", bufs=2, space="PSUM")
            )

            # K projections first (paced by the xb chunk DMAs), then Q,
            # then the V^T blocks computed DIRECTLY as xbT-block @ WvT in
            # bf16 (1 cyc/row even at ap=128): no separate V projection, no
            # PE transposes, no bias (bv is folded into xq_f host-side).
            xbvf = xbv.rearrange("c t s -> c (t s)")
            # matmul PSUM destinations must sit inside one 2KB bank, so
            # each 576-wide projection is two bank-aligned 288-wide matmuls
            # into a [C, 2, 512] tile; the pos-add reads the strided view.
            def proj(dst_full, sl, w, rhs, pzcol, post):
                pp = apsum.tile([C, 2, 512], F32, tag="kproj", name="pp")
                for j in range(2):
                    nc.tensor.matmul(
                        pp[:, j, 0:288], w, rhs[:, bass.ts(j, 288)],
                        start=True, stop=True,
                    )
                nc.vector.scalar_tensor_tensor(
                    out=dst_full[:, sl].rearrange("c (j s) -> c j s", j=2),
                    in0=pp[:, :, 0:288], scalar=pzcol,
                    in1=post.rearrange("c (j s) -> c j s", j=2),
                    op0=ALU.add, op1=ALU.add,
                )

            for t in range(T):
                proj(kT, bass.ts(t, HW), wk, xb[:, t, :], pzkt[:, t : t + 1], posyxt)
            for t in range(TPC):
                proj(qT, bass.ts(t, HW), wq, xq[:, t, :], pzqt[:, t : t + 1], posqxt)
            for g in range(NKB // 3):
                pvb = apsum.tile([C, 3, C], F32, tag="vbps")
                for j in range(3):
                    nk = 3 * g + j
                    nc.tensor.matmul(
                        pvb[:, j, :], xbvf[:, bass.ts(nk, C)], wv,
                        start=True, stop=True,
                    )
                nc.vector.tensor_copy(vb[:, 3 * g : 3 * g + 3, :], pvb)

        # ---------------- phase B + C interleaved emission ----------------
        # Attention (phase B) per q-tile; LayerNorm1 of image i is emitted as
        # soon as its attention outputs are normalized so it overlaps the
        # remaining attention on other engines; convs (phase C) follow.
        cctx = ExitStack()
        cpool = cctx.enter_context(tc.tile_pool(name="cpool", bufs=1))
        lnt = cctx.enter_context(tc.tile_pool(name="lnt", bufs=2))
        hidp = cctx.enter_context(tc.tile_pool(name="hidp", bufs=2))
        bctx = ExitStack()
        ptpool = bctx.enter_context(tc.tile_pool(name="ptpool", bufs=3))
        spsum = bctx.enter_context(tc.tile_pool(name="spsum", bufs=2, space="PSUM"))
        opsum = bctx.enter_context(tc.tile_pool(name="opsum", bufs=1, space="PSUM"))
        rpsum = bctx.enter_context(tc.tile_pool(name="rpsum", bufs=1, space="PSUM"))
        npool = bctx.enter_context(tc.tile_pool(name="npool", bufs=2))

        y_ln = cpool.tile([C, NQ], F32)

        def ln_stats(src):
            """channels-first LN stats on a [C, n] tile: returns (s1, varp).

            var*C = s2 - s1^2/C; final /C folded into the Sqrt scale in
            ln_finish.  ACT Square shares the Exp function set, so stats can
            interleave with attention without a table reload; only the Sqrt
            in ln_finish switches sets.  partition_all_reduce is chunked so
            it pipelines with the producer of src.
            """
            n = src.shape[-1]
            s1 = lnt.tile([C, HW], F32, tag="ln_s1", name="lns1")[:, 0:n]
            for h in range(n // 288):
                hs = bass.ts(h, 288)
                nc.gpsimd.partition_all_reduce(
                    s1[:, hs], src[:, hs], channels=C,
                    reduce_op=bass.bass_isa.ReduceOp.add,
                )
            sq = lnt.tile([C, HW], F32, tag="ln_sq", name="lnsq")[:, 0:n]
            nc.scalar.activation(sq, src, ACTF.Square)
            s2 = lnt.tile([C, HW], F32, tag="ln_s2", name="lns2")[:, 0:n]
            for h in range(n // 288):
                hs = bass.ts(h, 288)
                nc.gpsimd.partition_all_reduce(
                    s2[:, hs], sq[:, hs], channels=C,
                    reduce_op=bass.bass_isa.ReduceOp.add,
                )
            s1sq = lnt.tile([C, HW], F32, tag="ln_sq", name="lnsq")[:, 0:n]
            nc.vector.tensor_tensor(s1sq, s1, s1, op=ALU.mult)
            varp = lnt.tile([C, HW], F32, tag="ln_vp", name="lnvp")[:, 0:n]
            nc.vector.scalar_tensor_tensor(
                out=varp, in0=s1sq, scalar=-1.0 / C, in1=s2,
                op0=ALU.mult, op1=ALU.add,
            )
            return s1, varp

        def ln_finish(dst, src, s1, varp, wt, bt):
            n = src.shape[-1]
            sd = lnt.tile([C, HW], F32, tag="ln_s2", name="lns2")[:, 0:n]
            nc.scalar.activation(sd, varp, ACTF.Sqrt, bias=epst, scale=1.0 / C)
            inv = lnt.tile([C, HW], F32, tag="ln_vp", name="lnvp")[:, 0:n]
            nc.vector.reciprocal(inv, sd)
            yc = lnt.tile([C, HW], F32, tag="ln_sq", name="lnsq")[:, 0:n]
            nc.vector.scalar_tensor_tensor(
                out=yc, in0=s1, scalar=-1.0 / C, in1=src,
                op0=ALU.mult, op1=ALU.add,
            )
            yn = lnt.tile([C, HW], F32, tag="ln_s1", name="lns1")[:, 0:n]
            nc.vector.tensor_tensor(yn, yc, inv, op=ALU.mult)
            nc.vector.tensor_scalar(
                out=dst, in0=yn, scalar1=wt, scalar2=bt, op0=ALU.mult, op1=ALU.add
            )

        ln1_st = {}

        def emit_ln1_stats(img):
            isl = bass.ds(img * HW, HW)
            y_img = cpool.tile([C, HW], F32, tag=f"y{img}", name="yimg")
            nc.vector.tensor_tensor(
                y_img, ot_sb[:, isl], xqf[:, img, :], op=ALU.add
            )
            ln1_st[img] = (y_img, *ln_stats(y_img))

        def emit_ln1_finish(img):
            isl = bass.ds(img * HW, HW)
            y_img, s1, varp = ln1_st.pop(img)
            ln_finish(y_ln[:, isl], y_img, s1, varp, n1wt, n1bt)

        # ---- conv helpers (defined early; prep runs during attention) ----
        PW = W + 2  # column-padded image width (26)
        z_in = cpool.tile([C, NQ], F32)
        cps = {}
        convst = {}

        zpt = cpool.tile([C, H, 2], F32)
        nc.vector.memset(zpt, 0.0)

        def pad_cols(t_ap):
            """Zero the two pad columns (0 and PW-1) of a [C,H,PW] view.

            Strided memsets fail the walrus ISA check, so copy from a
            zeroed tile instead.
            """
            v = t_ap.rearrange("c (h w) -> c h w", w=PW)
            nc.vector.tensor_copy(v[:, :, 0:1], zpt[:, :, 0:1])
            nc.vector.tensor_copy(v[:, :, PW - 1 : PW], zpt[:, :, 1:2])

        def row_window(half, tap):
            """Row-clipped SAME-conv window for one 3x3 tap on a 12-row half.

            Output rows are clipped (PSUM dst stays contiguous); columns
            always valid thanks to the zero pad columns.  Returns
            (out_rows, in_rows, in_cols).
            """
            dy, dx = tap // 3, tap % 3
            y0 = max(12 * half, 1 - dy)
            y1 = min(12 * half + 12, H + 1 - dy)
            return (
                slice(y0 - 12 * half, y1 - 12 * half),
                slice(y0 + dy - 1, y1 + dy - 1),
                slice(dx, dx + W),
            )

        def emit_conv_prep(img):
            """Build the padded-image and hid tiles (DVE only)."""
            isl = bass.ds(img * HW, HW)
            yp = hidp.tile([C, H * PW], F32R, tag="ypad", name="ypad")
            pad_cols(yp)
            ypv = yp.rearrange("c (h w) -> c h w", w=PW)
            nc.vector.tensor_copy(
                ypv[:, :, 1 : W + 1],
                y_ln[:, isl].rearrange("c (h w) -> c h w", w=W),
            )
            hids = []
            for ch in range(4):
                hid = hidp.tile([C, H * PW], F32R, tag=f"hid{ch}", name="hid")
                pad_cols(hid)
                hids.append(hid)
            convst[img] = (ypv, hids)

        def emit_conv_mms(img):
            ypv, hids = convst.pop(img)
            # conv1 + bias + relu -> hids.  The 4 oc-chunks contract
            # disjoint 32-row ic slices -> 4 concurrent row-tiles on PE.
            for half in range(2):
                pss = [
                    cps["c1"].tile([C, 288], F32, tag=f"c1_{j}", name=f"c1ps{j}")
                    for j in range(4)
                ]
                for tap in range(9):
                    orows, irows, icols = row_window(half, tap)
                    for j in range(4):
                        psv = pss[j].rearrange("c (h w) -> c h w", w=W)
                        nc.tensor.matmul(
                            psv[:, orows, :],
                            w1t[32 * j : 32 * j + 32, tap, :],
                            ypv[32 * j : 32 * j + 32, irows, icols],
                            start=(tap == 0),
                            stop=(tap == 8),
                            tile_position=(32 * j, 0),
                        )
                for j in range(4):
                    hv = hids[j].rearrange("c (h w) -> c h w", w=PW)
                    nc.vector.tensor_scalar(
                        out=hv[:, bass.ts(half, 12), 1 : W + 1],
                        in0=pss[j],
                        scalar1=b1t[:, j : j + 1],
                        scalar2=0.0,
                        op0=ALU.add,
                        op1=ALU.max,
                    )
            # conv2 + bias + residual(y_ln): 4 ic-chunks accumulate into
            # one PSUM tile.
            for half in range(2):
                ps2 = cps["c2"].tile([C, 288], F32, tag="c2", name="c2ps")
                ps2v = ps2.rearrange("c (h w) -> c h w", w=W)
                for k in range(4):
                    hv = hids[k].rearrange("c (h w) -> c h w", w=PW)
                    for tap in range(9):
                        orows, irows, icols = row_window(half, tap)
                        nc.tensor.matmul(
                            ps2v[:, orows, :],
                            w2t[:, tap, k, :],
                            hv[:, irows, icols],
                            start=(tap == 0 and k == 0),
                            stop=(tap == 8 and k == 3),
                        )
                hsl = bass.ds(img * HW + half * 288, 288)
                nc.vector.scalar_tensor_tensor(
                    out=z_in[:, hsl],
                    in0=ps2,
                    scalar=b2t,
                    in1=y_ln[:, hsl],
                    op0=ALU.add,
                    op1=ALU.add,
                )

        def emit_ln2(img):
            # fully per-half pipelines: shortens the exposed tail latency
            for hh in range(2):
                hsl = bass.ds(img * HW + hh * 288, 288)
                zv = z_in[:, hsl]
                s1, varp = ln_stats(zv)
                z_h = cpool.tile([C, 288], F32, tag=f"zo{hh}", name="zh")
                ln_finish(z_h, zv, s1, varp, n2wt, n2bt)
                nc.sync.dma_start(out[img][:, bass.ts(hh, 288)], z_h)

        # phase B: software-pipelined attention
        for oq in range(NOQ):
            qsl = bass.ts(oq, NQT)
            ot_ps = opsum.tile([C, NQT], F32, tag="ot")
            rs_ps = rpsum.tile([1, NQT], F32, tag="rs")
            pts = {}

            def emit_pv(tri, ot_ps=ot_ps, rs_ps=rs_ps, pts=pts):
                pt = pts.pop(tri)
                for j in range(3):
                    nk = 3 * tri + j
                    nc.tensor.matmul(
                        ot_ps, vb[:, nk, :], pt[:, j, :],
                        start=(nk == 0), stop=(nk == NKB - 1),
                    )
                    nc.tensor.matmul(
                        rs_ps, onesr, pt[:, j, :],
                        start=(nk == 0), stop=(nk == NKB - 1),
                    )

            for tri in range(NTRI):
                st = spsum.tile([C, 3, 512], F32, tag="st")
                for j in range(3):
                    nk = 3 * tri + j
                    nc.tensor.matmul(
                        st[:, j, 0:NQT], kT[:, bass.ts(nk, C)], qT[:, qsl],
                        start=True, stop=True,
                    )
                pt = ptpool.tile([C, 3, NQT], F32R, tag="pt")
                nc.scalar.activation(pt, st[:, :, 0:NQT], ACTF.Exp)
                pts[tri] = pt
                if tri >= 1:
                    emit_pv(tri - 1)
            emit_pv(NTRI - 1)

            rinv = npool.tile([1, NQT], F32, tag="rinv")
            nc.vector.reciprocal(rinv, rs_ps)
            rb = npool.tile([C, NQT], F32, tag="rb")
            nc.gpsimd.partition_broadcast(rb, rinv)
            nc.vector.tensor_tensor(ot_sb[:, qsl], ot_ps, rb, op=ALU.mult)

            # LN1 overlap: image i is covered once q-tile i+1 is normalized.
            # Image 0's full LN (incl. the Sqrt table switch) runs between
            # oq1 and oq2 so conv(0) can start the moment attention ends.
            if oq == 1:
                emit_ln1_stats(0)
                emit_ln1_finish(0)
                emit_conv_prep(0)
            elif oq == 2:
                emit_ln1_stats(1)

        bctx.close()

        # ---------------- phase C: grouped-conv FFN + LN2 ----------------
        cps["c1"] = cctx.enter_context(tc.tile_pool(name="c1ps", bufs=1, space="PSUM"))
        cps["c2"] = cctx.enter_context(tc.tile_pool(name="c2ps", bufs=2, space="PSUM"))

        emit_conv_mms(0)
        emit_ln1_finish(1)
        emit_conv_prep(1)
        emit_ln2(0)
        emit_conv_mms(1)
        emit_ln2(1)
        cctx.close()


_CACHED_NC = None


def _get_nc():
    global _CACHED_NC
    if _CACHED_NC is None:
        _CACHED_NC = build_program()
    return _CACHED_NC


def make_in_maps(inputs: dict) -> list[dict]:
    x = np.asarray(inputs["x"], dtype=np.float32)
    Wq = np.asarray(inputs["Wq"], dtype=np.float32)
    bq = np.asarray(inputs["bq"], dtype=np.float32)
    Wk = np.asarray(inputs["Wk"], dtype=np.float32)
    bk = np.asarray(inputs["bk"], dtype=np.float32)
    Wv = np.asarray(inputs["Wv"], dtype=np.float32)
    bv_ = np.asarray(inputs["bv"], dtype=np.float32)
    conv1_w = np.asarray(inputs["conv1_w"], dtype=np.float32)
    conv1_b = np.asarray(inputs["conv1_b"], dtype=np.float32)
    conv2_w = np.asarray(inputs["conv2_w"], dtype=np.float32)
    conv2_b = np.asarray(inputs["conv2_b"], dtype=np.float32)

    pos_yx, pz = _pos_embed_np()
    isq = np.float32(1.0 / np.sqrt(np.float32(C)))

    # conv1 lhsT [ic, tap, oc_within_chunk]: oc chunk j=ic//32 implied —
    # chunk j's 128 oc (groups 8j..8j+8) contract exactly ic rows 32j..32j+32.
    w1_np = np.zeros((C, 9, C), dtype=np.float32)
    for oc in range(CH):
        g = oc // (CH // GROUPS)  # 16 oc per group
        for icg in range(C // GROUPS):  # 4 ic per group
            ic = g * (C // GROUPS) + icg
            w1_np[ic, :, oc % C] = conv1_w[oc, icg].reshape(9)
    # conv2 block-diagonal dense lhsT [ic%128, tap, ic//128, oc]
    w2_np = np.zeros((C, 9, 4, C), dtype=np.float32)
    for oc in range(C):
        g = oc // (C // GROUPS)  # 4 oc per group
        for icg in range(CH // GROUPS):  # 16 ic per group
            ic = g * (CH // GROUPS) + icg
            w2_np[ic % C, :, ic // C, oc] = conv2_w[oc, icg].reshape(9)

    shared = {
        "posyx": np.ascontiguousarray(pos_yx.T),
        "pzk": np.ascontiguousarray((pz + bk[None, :]).T),
        "posqx": np.ascontiguousarray(pos_yx.T) * isq,
        "wqT": np.ascontiguousarray(Wq.T) * isq,
        "wkT": np.ascontiguousarray(Wk.T),
        "wvT": np.ascontiguousarray(Wv.T).astype(ml_dtypes.bfloat16),
        "ones_r": np.ones((C, 1), dtype=np.float32),
        "w1": w1_np,
        "b1": np.ascontiguousarray(conv1_b.reshape(4, C).T),
        "b2": conv2_b[:, None],
        "n1w": np.asarray(inputs["norm1_w"], dtype=np.float32)[:, None],
        "n1b": np.asarray(inputs["norm1_b"], dtype=np.float32)[:, None],
        "n2w": np.asarray(inputs["norm2_w"], dtype=np.float32)[:, None],
        "n2b": np.asarray(inputs["norm2_b"], dtype=np.float32)[:, None],
        "w2": w2_np,
    }

    in_maps = []
    for j in range(NCORES):
        bi = j // 4
        t0 = TPC * (j % 4)
        xb_np = np.ascontiguousarray(x[bi].reshape(T, C, HW))
        xq_np = np.ascontiguousarray(x[bi, t0 : t0 + TPC].reshape(TPC, C, HW))
        pzq_np = np.ascontiguousarray(
            ((pz[t0 : t0 + TPC] + bq[None, :]) * isq).T
        )
        in_maps.append(
            {
                "xb_r": xb_np,
                "xb_bf": xb_np.astype(ml_dtypes.bfloat16),
                "xq_r": xq_np,
                # bv folded in: attention-out bias lands via the residual add
                "xq_f": xq_np + bv_[None, :, None],
                "pzq": pzq_np,
                **shared,
            }
        )
    return in_maps


def gather_output(results: list[dict]) -> np.ndarray:
    out = np.empty((B, T, C, H, W), dtype=np.float32)
    for j in range(NCORES):
        bi = j // 4
        t0 = TPC * (j % 4)
        out[bi, t0 : t0 + TPC] = results[j]["out"].reshape(TPC, C, H, W)
    return out


def kernel(**inputs) -> np.ndarray:
    nc = _get_nc()
    in_maps = make_in_maps(inputs)
    res = run_bass_kernel_spmd(nc, in_maps, list(range(NCORES)))
    return gather_output(res.results)
